# revision 5
# baseline (speedup 1.0000x reference)
"""nn_Critic Trainium2 kernel: 8-core SPMD via Bass/Tile.

Sharding: data-parallel over batch (64 rows/core). Conv stack + mb GEMM in
fp32r/bf16; minibatch-discrimination via AllGather of transposed activations
+ per-row |a_i - a_j| (DVE/ACT) + selector-matmul d-sum (PE) + fused
exp/row-sum (ACT accum_out).
"""
import numpy as np
import ml_dtypes

import concourse.bass as bass
import concourse.mybir as mybir
import concourse.tile as tile
from concourse import bacc
from concourse.bass_utils import run_bass_kernel_spmd
from concourse.masks import make_identity

dt = mybir.dt
F32 = dt.float32
F32R = dt.float32r
BF16 = dt.bfloat16
U16 = dt.uint16
Alu = mybir.AluOpType
Act = mybir.ActivationFunctionType

N, L, LEADS = 512, 640, 3
NC = 8
NB = N // NC  # 64 rows per core
L1, L2, L3 = 322, 161, 79
C1, C2, C3 = 64, 128, 256
NUM_K, D = 100, 5
FLAT = C3 * L3  # 20224
L1P = L1 + 6    # padded x1 row: cols 0..2 pad, 3..324 data, 325..327 pad
J2 = 84         # padded x2 (de-interleaved) row length per parity

_STATE = {}


def _sn(w, u):
    wm = w.reshape(w.shape[0], -1).astype(np.float32)
    v = wm.T @ u
    v = v / (np.linalg.norm(v) + 1e-12)
    u2 = wm @ v
    u2 = u2 / (np.linalg.norm(u2) + 1e-12)
    sigma = u2 @ (wm @ v)
    return (w / sigma).astype(np.float32)


def _build_program():
    nc = bacc.Bacc("TRN2", target_bir_lowering=False, debug=False, num_devices=NC)

    im2_d = nc.dram_tensor("im2", [15, NB * L1], F32, kind="ExternalInput").ap()
    w1_d = nc.dram_tensor("w1i", [15, 64], F32, kind="ExternalInput").ap()
    w2_d = nc.dram_tensor("w2p", [128, 512], BF16, kind="ExternalInput").ap()
    w3_d = nc.dram_tensor("w3t", [128, 2304], F32, kind="ExternalInput").ap()
    sel_d = nc.dram_tensor("sel", [125, 32], BF16, kind="ExternalInput").ap()
    fcwf_d = nc.dram_tensor("fcwf", [128, 2], F32, kind="ExternalInput").ap()
    mbw_d = nc.dram_tensor("mbw", [FLAT, 500], F32, kind="ExternalInput").ap()
    fcw2_d = nc.dram_tensor("fcw2", [FLAT, 2], F32, kind="ExternalInput").ap()
    condc_d = nc.dram_tensor("condc", [NB, 1], F32, kind="ExternalInput").ap()
    zeros_d = nc.dram_tensor("zeros", [128, 256], F32, kind="ExternalInput").ap()
    y_d = nc.dram_tensor("y", [NB, 1], F32, kind="ExternalOutput").ap()

    with tile.TileContext(nc) as tc:
        with (
            tc.tile_pool(name="w", bufs=1) as wp,
            tc.tile_pool(name="big", bufs=2) as bigp,
            tc.tile_pool(name="x1p", bufs=1) as x1p,
            tc.tile_pool(name="x2p", bufs=1) as x2p,
            tc.tile_pool(name="mbwp", bufs=6) as mbwp,
            tc.tile_pool(name="actp", bufs=1) as actp,
            tc.tile_pool(name="mbd", bufs=3) as mbdp,
            tc.tile_pool(name="psA", bufs=4, space="PSUM") as psA,
            tc.tile_pool(name="psB", bufs=1, space="PSUM") as psB,
            tc.tile_pool(name="psC", bufs=2, space="PSUM") as psC,
            tc.tile_pool(name="psD", bufs=1, space="PSUM") as psD,
            tc.tile_pool(name="dram", bufs=1, space="DRAM") as dramp,
        ):
            # ---- static weights / constants
            w1i = wp.tile([15, 64], F32R, tag="w1i")
            nc.sync.dma_start(out=w1i[:], in_=w1_d[:, :].bitcast(F32R))
            w2p = wp.tile([128, 512], BF16, tag="w2p")
            nc.sync.dma_start(out=w2p[:], in_=w2_d[:, :])
            w3t = wp.tile([128, 2304], F32R, tag="w3t")
            nc.sync.dma_start(out=w3t[:], in_=w3_d[:, :].bitcast(F32R))
            sel = wp.tile([125, 32], BF16, tag="sel")
            nc.sync.dma_start(out=sel[:], in_=sel_d[:, :])
            fcwf = wp.tile([128, 2], F32, tag="fcwf")
            nc.sync.dma_start(out=fcwf[:], in_=fcwf_d[:, :])
            condc = wp.tile([NB, 1], F32, tag="condc")
            nc.sync.dma_start(out=condc[:], in_=condc_d[:, :])
            ident = wp.tile([64, 64], F32, tag="ident")
            make_identity(nc, ident[:])
            alpha64 = wp.tile([64, 1], F32, tag="alpha64")
            nc.vector.memset(alpha64[:], 0.2)
            alpha128 = wp.tile([128, 1], F32, tag="alpha128")
            nc.vector.memset(alpha128[:], 0.2)

            # ---- persistent activations
            x1d = x1p.tile([128, NB * L1P], BF16, tag="x1d")
            nc.gpsimd.memset(x1d[:], 0.0)
            x2e = x2p.tile([128, NB * J2], F32R, tag="x2e")
            x2o = x2p.tile([128, NB * J2], F32R, tag="x2o")
            x2ev0 = x2e[:].rearrange("p (n j) -> p n j", j=J2)
            x2ov0 = x2o[:].rearrange("p (n j) -> p n j", j=J2)
            nc.sync.dma_start(
                out=x2ev0[:, :, 0:1], in_=zeros_d[:, 0:NB].bitcast(F32R)
            )
            nc.sync.dma_start(
                out=x2ev0[:, :, 82:84], in_=zeros_d[:, 0 : 2 * NB].bitcast(F32R)
            )
            nc.sync.dma_start(
                out=x2ov0[:, :, 0:1], in_=zeros_d[:, 0:NB].bitcast(F32R)
            )
            nc.sync.dma_start(
                out=x2ov0[:, :, 81:84], in_=zeros_d[:, 0 : 3 * NB].bitcast(F32R)
            )

            def leaky(idx, out_ap, in_ap, alpha):
                # alternate engines: ACT Prelu vs DVE max(0.2x, x)
                if idx % 3 != 2:
                    nc.scalar.activation(
                        out=out_ap, in_=in_ap, func=Act.Prelu, scale=1.0,
                        alpha=alpha,
                    )
                else:
                    P, F = in_ap.shape[0], in_ap.free_size()
                    tmp = mbdp.tile([128, 512], F32, tag="lk_tmp", name=f"lk{idx}_{P}")
                    tv = tmp[0:P, 0:F]
                    nc.vector.tensor_scalar(
                        out=tv, in0=in_ap, scalar1=0.2, scalar2=None, op0=Alu.mult
                    )
                    nc.vector.tensor_max(out_ap, tv, in_ap)

            # ================= conv1 (fp32r, im2col K=15) =================
            for q in range(4):  # chunks of 16 batch rows
                ch = bigp.tile([15, 16 * L1], F32R, tag="big")
                nc.sync.dma_start(
                    out=ch[:], in_=im2_d[:, q * 16 * L1 : (q + 1) * 16 * L1].bitcast(F32R)
                )
                for j in range(16):
                    n = q * 16 + j
                    pc1 = psA.tile([64, L1], F32, tag="cps")
                    nc.tensor.matmul(
                        pc1[:], lhsT=w1i[:], rhs=ch[:, j * L1 : (j + 1) * L1],
                        start=True, stop=True,
                    )
                    leaky(n, x1d[0:64, n * L1P + 3 : n * L1P + 3 + L1], pc1[:], alpha64[:])
            # duplicate lower half shifted by +1 col (tap pairing)
            nc.sync.dma_start(
                out=x1d[64:128, 0 : NB * L1P - 1], in_=x1d[0:64, 1 : NB * L1P]
            )

            # ================= conv2 (bf16, tap pairs K=128) =================
            # chunks of 3 n (psum [128, 483]); last chunk 1 n
            chunks2 = [(i * 3, 3) for i in range(21)] + [(63, 1)]
            x1v = x1d[:].rearrange("p (n c) -> p n c", c=L1P)
            for g0 in range(0, len(chunks2), 4):
                grp = chunks2[g0 : g0 + 4]
                pcs = []
                for (n0, cnt) in grp:
                    pcs.append(psA.tile([128, cnt * L2], F32, tag="cps", name=f"pc2_{g0}_{n0}"))
                for p in range(4):
                    for (n0, cnt), pc2 in zip(grp, pcs):
                        rhs = x1v[:, n0 : n0 + cnt, 2 * p : 2 * p + 2 * L2 : 2]
                        nc.tensor.matmul(
                            pc2[:], lhsT=w2p[:, p * 128 : (p + 1) * 128], rhs=rhs,
                            start=(p == 0), stop=(p == 3),
                        )
                for gi, ((n0, cnt), pc2) in enumerate(zip(grp, pcs)):
                    pv = pc2[:].rearrange("p (n l) -> p n l", l=L2)
                    x2ev = x2e[:].rearrange("p (n j) -> p n j", j=J2)
                    x2ov = x2o[:].rearrange("p (n j) -> p n j", j=J2)
                    leaky(
                        g0 + gi,
                        x2ev[:, n0 : n0 + cnt, 1 : 82],
                        pv[:, :, 0:L2:2],
                        alpha128[:],
                    )
                    leaky(
                        g0 + gi + 1,
                        x2ov[:, n0 : n0 + cnt, 1 : 81],
                        pv[:, :, 1:L2:2],
                        alpha128[:],
                    )

            # ================= conv3 (fp32r, 9 taps K=128, 2 co-tiles) =======
            x3 = []
            for ct in range(2):
                x3.append(bigp.tile([128, NB * L3], F32R, tag="big", name=f"x3_{ct}"))
            x2ev = x2e[:].rearrange("p (n j) -> p j n", j=J2)
            x2ov = x2o[:].rearrange("p (n j) -> p j n", j=J2)
            chunks3 = [(i * 6, 6) for i in range(10)] + [(60, 4)]
            for ct in range(2):
                for g0 in range(0, len(chunks3), 4):
                    grp = chunks3[g0 : g0 + 4]
                    pcs = []
                    for (n0, cnt) in grp:
                        pcs.append(psA.tile([128, L3 * cnt], F32, tag="cps", name=f"pc3_{ct}_{g0}_{n0}"))
                    for t in range(9):
                        lhsT = w3t[:, t * 256 + ct * 128 : t * 256 + (ct + 1) * 128]
                        src = x2ev if t % 2 == 0 else x2ov
                        toff = t // 2 if t % 2 == 0 else (t - 1) // 2
                        for (n0, cnt), pc3 in zip(grp, pcs):
                            # dims: (l3 outer stride1-in-j, n inner stride J2)
                            rhs = src[:, toff : toff + L3, n0 : n0 + cnt]
                            nc.tensor.matmul(
                                pc3[:], lhsT=lhsT, rhs=rhs,
                                start=(t == 0), stop=(t == 8),
                            )
                    for gi, ((n0, cnt), pc3) in enumerate(zip(grp, pcs)):
                        outv = x3[ct][:].rearrange("p (n l) -> p l n", l=L3)
                        leaky(
                            ct * 11 + g0 + gi,
                            outv[:, :, n0 : n0 + cnt],
                            pc3[:],
                            alpha128[:],
                        )

            # ================= mb GEMM + fc-flat (fp32r) ====================
            mbw_v = mbw_d.rearrange("(a b) n -> a b n", b=L3)
            fcw2_v = fcw2_d.rearrange("(a b) n -> a b n", b=L3)
            ps_act = psB.tile([64, 502], F32, tag="mbact")
            for ch_i in range(158):
                ct, l = ch_i // L3, ch_i % L3
                rhs_t = mbwp.tile([128, 502], F32R, tag="mbw")
                nc.sync.dma_start(
                    out=rhs_t[:, 0:500],
                    in_=mbw_v[ct * 128 : (ct + 1) * 128, l, :].bitcast(F32R),
                )
                nc.sync.dma_start(
                    out=rhs_t[:, 500:502],
                    in_=fcw2_v[ct * 128 : (ct + 1) * 128, l, :].bitcast(F32R),
                )
                lhsT = x3[ct][:].rearrange("p (n l) -> p l n", l=L3)[:, l, :]
                nc.tensor.matmul(
                    ps_act[:], lhsT=lhsT, rhs=rhs_t[:],
                    start=(ch_i == 0), stop=(ch_i == 157),
                )

            # ================= act transpose + AllGather ====================
            act_sb = actp.tile([64, 500], F32, tag="act_sb")
            nc.vector.tensor_copy(act_sb[:], ps_act[:, 0:500])
            fcflat = actp.tile([64, 1], F32, tag="fcflat")
            nc.vector.tensor_copy(fcflat[:], ps_act[:, 500:501])

            bounce = dramp.tile([500, 64], F32)
            gathered = dramp.tile([NC, 500, 64], F32)
            sct = []
            sctn = []
            for s in range(4):
                pt = psD.tile([125, 64], F32, tag="pt")
                nc.tensor.transpose(pt[:], act_sb[:, 125 * s : 125 * (s + 1)], ident[:])
                t_ = actp.tile([125, 64], F32, tag=f"sct{s}", name=f"sct{s}")
                nc.vector.tensor_copy(t_[:], pt[:])
                sct.append(t_)
                tn = actp.tile([125, 64], F32, tag=f"sctn{s}", name=f"sctn{s}")
                nc.vector.tensor_scalar(
                    out=tn[:], in0=t_[:], scalar1=-1.0, scalar2=None, op0=Alu.mult
                )
                sctn.append(tn)
                nc.sync.dma_start(out=bounce[125 * s : 125 * (s + 1), :], in_=t_[:])
            nc.gpsimd.collective_compute(
                "AllGather", Alu.bypass,
                replica_groups=[list(range(NC))],
                ins=[bounce[:].opt()], outs=[gathered[:].opt()],
            )
            act_bf = []
            for c in range(4):
                af = actp.tile([125, 512], F32, tag=f"actf{c}", name=f"actf{c}")
                src = gathered[:, 125 * c : 125 * (c + 1), :].rearrange(
                    "r p j -> p r j"
                )
                nc.sync.dma_start(
                    out=af[:].rearrange("p (r j) -> p r j", r=NC), in_=src
                )
                ab = actp.tile([125, 512], BF16, tag=f"actb{c}", name=f"actb{c}")
                nc.vector.tensor_copy(ab[:], af[:])
                act_bf.append(ab)

            # ================= MBD: per-row pairwise L1 + exp-sum ===========
            featsT = actp.tile([128, 64], F32, tag="featsT")
            for i in range(NB):
                act_c = 3 if i % 2 == 0 else 0
                ads = []
                for c in range(4):
                    if c == act_c:
                        ad = mbdp.tile([125, 512], BF16, tag="ad_act")
                        nc.scalar.activation(
                            out=ad[:], in_=act_bf[c][:], func=Act.Abs,
                            bias=sctn[c][:, i : i + 1], scale=1.0,
                        )
                    else:
                        d_ = mbdp.tile([125, 512], BF16, tag="d_tmp")
                        nc.vector.tensor_scalar(
                            out=d_[:], in0=act_bf[c][:],
                            scalar1=sct[c][:, i : i + 1], scalar2=None,
                            op0=Alu.subtract,
                        )
                        ad = mbdp.tile([125, 512], BF16, tag="ad_dve")
                        nc.vector.tensor_scalar(
                            out=ad[:].bitcast(U16), in0=d_[:].bitcast(U16),
                            scalar1=0x7FFF, scalar2=None, op0=Alu.bitwise_and,
                        )
                    ads.append(ad)
                l1p_t = psC.tile([128, 512], F32, tag="l1p")
                for c in range(4):
                    nc.tensor.matmul(
                        l1p_t[32 * c : 32 * c + 32, :], lhsT=sel[:], rhs=ads[c][:],
                        start=True, stop=True,
                        tile_position=(0, 96) if c == 3 else None,
                    )
                esc = mbdp.tile([128, 512], BF16, tag="esc")
                nc.scalar.activation(
                    out=esc[:], in_=l1p_t[:], func=Act.Exp, scale=-1.0,
                    accum_out=featsT[:, i : i + 1],
                )

            # ================= final fc =====================================
            ps_fc = psD.tile([64, 2], F32, tag="pt")
            nc.tensor.matmul(ps_fc[:], lhsT=featsT[:], rhs=fcwf[:], start=True, stop=True)
            t1 = actp.tile([64, 1], F32, tag="t1")
            nc.vector.tensor_add(t1[:], ps_fc[:, 0:1], fcflat[:])
            out_sb = actp.tile([64, 1], F32, tag="out_sb")
            nc.vector.tensor_add(out_sb[:], t1[:], condc[:])
            nc.sync.dma_start(out=y_d[:, :], in_=out_sb[:])

    nc.compile()
    return nc


def _host_prep(inputs):
    ecg = np.asarray(inputs["ecg"], dtype=np.float32)
    condition = np.asarray(inputs["condition"]).astype(np.int64)
    w1 = _sn(np.asarray(inputs["w1"], np.float32), np.asarray(inputs["u1"], np.float32))
    w2 = _sn(np.asarray(inputs["w2"], np.float32), np.asarray(inputs["u2"], np.float32))
    w3 = _sn(np.asarray(inputs["w3"], np.float32), np.asarray(inputs["u3"], np.float32))
    for bn in ("b1", "b2", "b3"):
        assert not np.any(np.asarray(inputs[bn])), "nonzero conv bias unsupported"
    emb = np.asarray(inputs["emb"], np.float32)
    cond_w = np.asarray(inputs["cond_w"], np.float32)
    cond_b = np.asarray(inputs["cond_b"], np.float32)
    mb_w = np.ascontiguousarray(np.asarray(inputs["mb_w"], np.float32))
    fc_w = np.asarray(inputs["fc_w"], np.float32)
    fc_b = np.asarray(inputs["fc_b"], np.float32)

    # conv1 im2col: rows (ci*5 + t), cols (n*322 + l): x[n, 2l+t-4, ci]
    xpad = np.zeros((N, L + 8, LEADS), np.float32)
    xpad[:, 4 : 4 + L, :] = ecg
    im2 = np.empty((15, N, L1), np.float32)
    for ci in range(LEADS):
        for t in range(5):
            im2[ci * 5 + t] = xpad[:, t : t + 2 * L1 : 2, ci]
    w1i = np.ascontiguousarray(w1.transpose(1, 2, 0).reshape(15, 64))

    w2p = np.zeros((128, 4, 128), np.float32)
    for p in range(4):
        w2p[0:64, p, :] = w2[:, :, 2 * p].T
        if 2 * p + 1 < 7:
            w2p[64:128, p, :] = w2[:, :, 2 * p + 1].T
    w2p = w2p.reshape(128, 512).astype(ml_dtypes.bfloat16)

    w3t = np.ascontiguousarray(w3.transpose(1, 2, 0).reshape(128, 2304))

    sel = np.zeros((125, 32), np.float32)
    for kd in range(125):
        sel[kd, kd // 5] = 1.0
    sel = sel.astype(ml_dtypes.bfloat16)

    fcwf = np.zeros((128, 2), np.float32)
    for c in range(4):
        fcwf[32 * c : 32 * c + 25, 0] = fc_w[0, FLAT + 25 * c : FLAT + 25 * c + 25]

    fcw2 = np.stack([fc_w[0, :FLAT], fc_w[0, :FLAT]], axis=1).astype(np.float32)
    fcw2 = np.ascontiguousarray(fcw2)

    cemb = emb[condition[:, 0]]
    cond = cemb @ cond_w.T + cond_b
    condc = (cond @ fc_w[0, FLAT + 100 :] + fc_b[0]).astype(np.float32)  # [512]

    in_maps = []
    for c in range(NC):
        rows = slice(c * NB, (c + 1) * NB)
        in_maps.append({
            "im2": np.ascontiguousarray(im2[:, rows, :].reshape(15, NB * L1)),
            "w1i": w1i,
            "w2p": w2p,
            "w3t": w3t,
            "sel": sel,
            "fcwf": fcwf,
            "mbw": mb_w,
            "fcw2": fcw2,
            "condc": np.ascontiguousarray(condc[rows].reshape(NB, 1)),
            "zeros": np.zeros((128, 256), np.float32),
        })
    return in_maps


def _run(inputs, trace=False):
    if "nc" not in _STATE:
        _STATE["nc"] = _build_program()
    in_maps = _host_prep(inputs)
    res = run_bass_kernel_spmd(_STATE["nc"], in_maps, list(range(NC)), trace=trace)
    out = np.concatenate([res.results[c]["y"] for c in range(NC)], axis=0)
    return out.astype(np.float32), res


def kernel(**inputs) -> np.ndarray:
    out, _ = _run(inputs, trace=False)
    return out


# revision 7
# speedup vs baseline: 1.3684x; 1.3684x over previous
"""nn_Critic Trainium2 kernel: 8-core SPMD via Bass/Tile.

Sharding: data-parallel over batch (64 rows/core). Conv stack + mb GEMM in
fp32r/bf16; minibatch-discrimination via AllGather of transposed activations
+ per-row |a_i - a_j| (DVE/ACT) + selector-matmul d-sum (PE) + fused
exp/row-sum (ACT accum_out).
"""
import numpy as np
import ml_dtypes

import concourse.bass as bass
import concourse.mybir as mybir
import concourse.tile as tile
from concourse import bacc
from concourse.bass_utils import run_bass_kernel_spmd
from concourse.masks import make_identity

dt = mybir.dt
F32 = dt.float32
F32R = dt.float32r
BF16 = dt.bfloat16
U16 = dt.uint16
Alu = mybir.AluOpType
Act = mybir.ActivationFunctionType

N, L, LEADS = 512, 640, 3
NC = 8
NB = N // NC  # 64 rows per core
L1, L2, L3 = 322, 161, 79
C1, C2, C3 = 64, 128, 256
NUM_K, D = 100, 5
FLAT = C3 * L3  # 20224
L1P = L1 + 6    # padded x1 row: cols 0..2 pad, 3..324 data, 325..327 pad
J2 = 84         # padded x2 (de-interleaved) row length per parity

_STATE = {}


def _sn(w, u):
    wm = w.reshape(w.shape[0], -1).astype(np.float32)
    v = wm.T @ u
    v = v / (np.linalg.norm(v) + 1e-12)
    u2 = wm @ v
    u2 = u2 / (np.linalg.norm(u2) + 1e-12)
    sigma = u2 @ (wm @ v)
    return (w / sigma).astype(np.float32)


def _build_program():
    nc = bacc.Bacc("TRN2", target_bir_lowering=False, debug=False, num_devices=NC)

    im2_d = nc.dram_tensor("im2", [15, NB * L1], F32, kind="ExternalInput").ap()
    w1_d = nc.dram_tensor("w1i", [15, 64], F32, kind="ExternalInput").ap()
    w2_d = nc.dram_tensor("w2p", [128, 512], BF16, kind="ExternalInput").ap()
    w3_d = nc.dram_tensor("w3t", [128, 2304], F32, kind="ExternalInput").ap()
    sel_d = nc.dram_tensor("sel", [125, 32], BF16, kind="ExternalInput").ap()
    fcwf_d = nc.dram_tensor("fcwf", [128, 2], F32, kind="ExternalInput").ap()
    mbw_d = nc.dram_tensor("mbw", [FLAT, 502], F32, kind="ExternalInput").ap()
    condc_d = nc.dram_tensor("condc", [NB, 1], F32, kind="ExternalInput").ap()
    zeros_d = nc.dram_tensor("zeros", [128, 256], F32, kind="ExternalInput").ap()
    y_d = nc.dram_tensor("y", [NB, 1], F32, kind="ExternalOutput").ap()

    with tile.TileContext(nc) as tc:
        with (
            tc.tile_pool(name="w", bufs=1) as wp,
            tc.tile_pool(name="big", bufs=2) as bigp,
            tc.tile_pool(name="x1p", bufs=1) as x1p,
            tc.tile_pool(name="x2p", bufs=1) as x2p,
            tc.tile_pool(name="mbwp", bufs=6) as mbwp,
            tc.tile_pool(name="actp", bufs=1) as actp,
            tc.tile_pool(name="mbd", bufs=3) as mbdp,
            tc.tile_pool(name="psA", bufs=4, space="PSUM") as psA,
            tc.tile_pool(name="psB", bufs=1, space="PSUM") as psB,
            tc.tile_pool(name="psC", bufs=2, space="PSUM") as psC,
            tc.tile_pool(name="psD", bufs=1, space="PSUM") as psD,
            tc.tile_pool(name="dram", bufs=1, space="DRAM") as dramp,
        ):
            # ---- static weights / constants
            w1i = wp.tile([15, 64], F32R, tag="w1i")
            nc.sync.dma_start(out=w1i[:], in_=w1_d[:, :].bitcast(F32R))
            w2p = wp.tile([128, 512], BF16, tag="w2p")
            nc.sync.dma_start(out=w2p[:], in_=w2_d[:, :])
            w3t = wp.tile([128, 2304], F32R, tag="w3t")
            nc.sync.dma_start(out=w3t[:], in_=w3_d[:, :].bitcast(F32R))
            sel = wp.tile([125, 32], BF16, tag="sel")
            nc.sync.dma_start(out=sel[:], in_=sel_d[:, :])
            fcwf = wp.tile([128, 2], F32, tag="fcwf")
            nc.sync.dma_start(out=fcwf[:], in_=fcwf_d[:, :])
            condc = wp.tile([NB, 1], F32, tag="condc")
            nc.sync.dma_start(out=condc[:], in_=condc_d[:, :])
            ident = wp.tile([64, 64], F32, tag="ident")
            make_identity(nc, ident[:])
            alpha64 = wp.tile([64, 1], F32, tag="alpha64")
            nc.vector.memset(alpha64[:], 0.2)
            alpha128 = wp.tile([128, 1], F32, tag="alpha128")
            nc.vector.memset(alpha128[:], 0.2)

            # ---- persistent activations
            x1d = x1p.tile([128, NB * L1P], BF16, tag="x1d")
            x1dv = x1d[:].rearrange("p (n c) -> p n c", c=L1P)
            nc.gpsimd.memset(x1dv[:, :, 0:3], 0.0)
            nc.gpsimd.memset(x1dv[:, :, 3 + L1 : L1P], 0.0)
            x2e = x2p.tile([128, NB * J2], F32R, tag="x2e")
            x2o = x2p.tile([128, NB * J2], F32R, tag="x2o")
            x2ev0 = x2e[:].rearrange("p (n j) -> p n j", j=J2)
            x2ov0 = x2o[:].rearrange("p (n j) -> p n j", j=J2)
            nc.gpsimd.dma_start(
                out=x2ev0[:, :, 0:1], in_=zeros_d[:, 0:NB].bitcast(F32R)
            )
            nc.gpsimd.dma_start(
                out=x2ev0[:, :, 82:84], in_=zeros_d[:, 0 : 2 * NB].bitcast(F32R)
            )
            nc.gpsimd.dma_start(
                out=x2ov0[:, :, 0:1], in_=zeros_d[:, 0:NB].bitcast(F32R)
            )
            nc.gpsimd.dma_start(
                out=x2ov0[:, :, 81:84], in_=zeros_d[:, 0 : 3 * NB].bitcast(F32R)
            )

            def leaky(idx, out_ap, in_ap, alpha):
                # alternate engines: ACT Prelu vs DVE max(0.2x, x)
                if idx % 3 != 2:
                    nc.scalar.activation(
                        out=out_ap, in_=in_ap, func=Act.Prelu, scale=1.0,
                        alpha=alpha,
                    )
                else:
                    P, F = in_ap.shape[0], in_ap.free_size()
                    tmp = mbdp.tile([128, 512], F32, tag="lk_tmp", name=f"lk{idx}_{P}")
                    tv = tmp[0:P, 0:F]
                    nc.vector.tensor_scalar(
                        out=tv, in0=in_ap, scalar1=0.2, scalar2=None, op0=Alu.mult
                    )
                    nc.vector.tensor_max(out_ap, tv, in_ap)

            # ================= conv1 (fp32r, im2col K=15) =================
            for q in range(4):  # chunks of 16 batch rows
                ch = bigp.tile([15, 16 * L1], F32R, tag="big")
                nc.sync.dma_start(
                    out=ch[:], in_=im2_d[:, q * 16 * L1 : (q + 1) * 16 * L1].bitcast(F32R)
                )
                for j in range(16):
                    n = q * 16 + j
                    pc1 = psA.tile([64, L1], F32, tag="cps")
                    nc.tensor.matmul(
                        pc1[:], lhsT=w1i[:], rhs=ch[:, j * L1 : (j + 1) * L1],
                        start=True, stop=True,
                    )
                    leaky(n, x1d[0:64, n * L1P + 3 : n * L1P + 3 + L1], pc1[:], alpha64[:])
            # duplicate lower half shifted by +1 col (tap pairing)
            nc.sync.dma_start(
                out=x1d[64:128, 0 : NB * L1P - 1], in_=x1d[0:64, 1 : NB * L1P]
            )

            # ================= conv2 (bf16, tap pairs K=128) =================
            # chunks of 3 n (psum [128, 483]); last chunk 1 n
            chunks2 = [(i * 3, 3) for i in range(21)] + [(63, 1)]
            x1v = x1d[:].rearrange("p (n c) -> p n c", c=L1P)
            for g0 in range(0, len(chunks2), 4):
                grp = chunks2[g0 : g0 + 4]
                pcs = []
                for (n0, cnt) in grp:
                    pcs.append(psA.tile([128, cnt * L2], F32, tag="cps", name=f"pc2_{g0}_{n0}"))
                for p in range(4):
                    for (n0, cnt), pc2 in zip(grp, pcs):
                        rhs = x1v[:, n0 : n0 + cnt, 2 * p : 2 * p + 2 * L2 : 2]
                        nc.tensor.matmul(
                            pc2[:], lhsT=w2p[:, p * 128 : (p + 1) * 128], rhs=rhs,
                            start=(p == 0), stop=(p == 3),
                        )
                for gi, ((n0, cnt), pc2) in enumerate(zip(grp, pcs)):
                    pv = pc2[:].rearrange("p (n l) -> p n l", l=L2)
                    x2ev = x2e[:].rearrange("p (n j) -> p n j", j=J2)
                    x2ov = x2o[:].rearrange("p (n j) -> p n j", j=J2)
                    leaky(
                        g0 + gi,
                        x2ev[:, n0 : n0 + cnt, 1 : 82],
                        pv[:, :, 0:L2:2],
                        alpha128[:],
                    )
                    leaky(
                        g0 + gi + 1,
                        x2ov[:, n0 : n0 + cnt, 1 : 81],
                        pv[:, :, 1:L2:2],
                        alpha128[:],
                    )

            # ================= conv3 (fp32r, 9 taps K=128, 2 co-tiles) =======
            x3 = []
            for ct in range(2):
                x3.append(bigp.tile([128, NB * L3], F32R, tag="big", name=f"x3_{ct}"))
            x2ev = x2e[:].rearrange("p (n j) -> p j n", j=J2)
            x2ov = x2o[:].rearrange("p (n j) -> p j n", j=J2)
            chunks3 = [(i * 6, 6) for i in range(10)] + [(60, 4)]
            mbw_v = mbw_d.rearrange("(a b) n -> a b n", b=L3)
            ps_act = psB.tile([64, 502], F32, tag="mbact")
            lgroups = [(i * 2, 2) for i in range(39)] + [(78, 1)]
            qi = 0
            for ct in range(2):
                for g0 in range(0, len(chunks3), 4):
                    grp = chunks3[g0 : g0 + 4]
                    pcs = []
                    for (n0, cnt) in grp:
                        pcs.append(psA.tile([128, L3 * cnt], F32, tag="cps", name=f"pc3_{ct}_{g0}_{n0}"))
                    for t in range(9):
                        lhsT = w3t[:, t * 256 + ct * 128 : t * 256 + (ct + 1) * 128]
                        src = x2ev if t % 2 == 0 else x2ov
                        toff = t // 2 if t % 2 == 0 else (t - 1) // 2
                        for (n0, cnt), pc3 in zip(grp, pcs):
                            # dims: (l3 outer stride1-in-j, n inner stride J2)
                            rhs = src[:, toff : toff + L3, n0 : n0 + cnt]
                            nc.tensor.matmul(
                                pc3[:], lhsT=lhsT, rhs=rhs,
                                start=(t == 0), stop=(t == 8),
                            )
                    for gi, ((n0, cnt), pc3) in enumerate(zip(grp, pcs)):
                        outv = x3[ct][:].rearrange("p (n l) -> p l n", l=L3)
                        leaky(
                            ct * 11 + g0 + gi,
                            outv[:, :, n0 : n0 + cnt],
                            pc3[:],
                            alpha128[:],
                        )
                # mb GEMM chunks for this co-tile (fused fc-flat column)
                xv = x3[ct][:].rearrange("p (n l) -> p l n", l=L3)
                for (l0, lc) in lgroups:
                    rhs_t = mbwp.tile([128, 2 * 502], F32R, tag="mbw", name=f"mbw{ct}_{l0}")
                    eng = nc.sync if qi % 2 == 0 else nc.scalar
                    eng.dma_start(
                        out=rhs_t[:, 0 : lc * 502],
                        in_=mbw_v[ct * 128 : (ct + 1) * 128, l0 : l0 + lc, :].bitcast(F32R),
                    )
                    qi += 1
                    for i_l in range(lc):
                        l = l0 + i_l
                        nc.tensor.matmul(
                            ps_act[:], lhsT=xv[:, l, :],
                            rhs=rhs_t[:, i_l * 502 : (i_l + 1) * 502],
                            start=(ct == 0 and l == 0), stop=(ct == 1 and l == 78),
                        )



            # ================= act transpose + AllGather ====================
            act_sb = actp.tile([64, 500], F32, tag="act_sb")
            nc.vector.tensor_copy(act_sb[:], ps_act[:, 0:500])
            fcflat = actp.tile([64, 1], F32, tag="fcflat")
            nc.vector.tensor_copy(fcflat[:], ps_act[:, 500:501])

            bounce = dramp.tile([500, 64], BF16)
            gathered = dramp.tile([NC, 500, 64], BF16)
            sct = []
            sctn = []
            for s in range(4):
                pt = psD.tile([125, 64], F32, tag="pt")
                nc.tensor.transpose(pt[:], act_sb[:, 125 * s : 125 * (s + 1)], ident[:])
                t_ = actp.tile([125, 64], F32, tag=f"sct{s}", name=f"sct{s}")
                nc.vector.tensor_copy(t_[:], pt[:])
                sct.append(t_)
                tn = actp.tile([125, 64], F32, tag=f"sctn{s}", name=f"sctn{s}")
                nc.vector.tensor_scalar(
                    out=tn[:], in0=t_[:], scalar1=-1.0, scalar2=None, op0=Alu.mult
                )
                sctn.append(tn)
                tb = actp.tile([125, 64], BF16, tag=f"sctb{s}", name=f"sctb{s}")
                nc.vector.tensor_copy(tb[:], pt[:])
                nc.sync.dma_start(out=bounce[125 * s : 125 * (s + 1), :], in_=tb[:])
            nc.gpsimd.collective_compute(
                "AllGather", Alu.bypass,
                replica_groups=[list(range(NC))],
                ins=[bounce[:].opt()], outs=[gathered[:].opt()],
            )
            act_bf = []
            for c in range(4):
                ab = actp.tile([125, 512], BF16, tag=f"actb{c}", name=f"actb{c}")
                src = gathered[:, 125 * c : 125 * (c + 1), :].rearrange(
                    "r p j -> p r j"
                )
                nc.sync.dma_start(
                    out=ab[:].rearrange("p (r j) -> p r j", r=NC), in_=src
                )
                act_bf.append(ab)

            # ================= MBD: per-row pairwise L1 + exp-sum ===========
            featsT = actp.tile([128, 64], F32, tag="featsT")
            for i in range(NB):
                act_c = 3 if i % 2 == 0 else 0
                ads = []
                for c in range(4):
                    if c == act_c:
                        ad = mbdp.tile([125, 512], BF16, tag="ad_act")
                        nc.scalar.activation(
                            out=ad[:], in_=act_bf[c][:], func=Act.Abs,
                            bias=sctn[c][:, i : i + 1], scale=1.0,
                        )
                    else:
                        d_ = mbdp.tile([125, 512], BF16, tag="d_tmp")
                        nc.vector.tensor_scalar(
                            out=d_[:], in0=act_bf[c][:],
                            scalar1=sct[c][:, i : i + 1], scalar2=None,
                            op0=Alu.subtract,
                        )
                        ad = mbdp.tile([125, 512], BF16, tag="ad_dve")
                        nc.vector.tensor_scalar(
                            out=ad[:].bitcast(U16), in0=d_[:].bitcast(U16),
                            scalar1=0x7FFF, scalar2=None, op0=Alu.bitwise_and,
                        )
                    ads.append(ad)
                l1p_t = psC.tile([128, 512], F32, tag="l1p")
                for c in range(4):
                    nc.tensor.matmul(
                        l1p_t[32 * c : 32 * c + 32, :], lhsT=sel[:], rhs=ads[c][:],
                        start=True, stop=True,
                        tile_position=(0, 96) if c == 3 else None,
                    )
                esc = mbdp.tile([128, 512], BF16, tag="esc")
                nc.scalar.activation(
                    out=esc[:], in_=l1p_t[:], func=Act.Exp, scale=-1.0,
                    accum_out=featsT[:, i : i + 1],
                )

            # ================= final fc =====================================
            ps_fc = psD.tile([64, 2], F32, tag="pt")
            nc.tensor.matmul(ps_fc[:], lhsT=featsT[:], rhs=fcwf[:], start=True, stop=True)
            t1 = actp.tile([64, 1], F32, tag="t1")
            nc.vector.tensor_add(t1[:], ps_fc[:, 0:1], fcflat[:])
            out_sb = actp.tile([64, 1], F32, tag="out_sb")
            nc.vector.tensor_add(out_sb[:], t1[:], condc[:])
            nc.sync.dma_start(out=y_d[:, :], in_=out_sb[:])

    nc.compile()
    return nc


def _host_prep(inputs):
    ecg = np.asarray(inputs["ecg"], dtype=np.float32)
    condition = np.asarray(inputs["condition"]).astype(np.int64)
    w1 = _sn(np.asarray(inputs["w1"], np.float32), np.asarray(inputs["u1"], np.float32))
    w2 = _sn(np.asarray(inputs["w2"], np.float32), np.asarray(inputs["u2"], np.float32))
    w3 = _sn(np.asarray(inputs["w3"], np.float32), np.asarray(inputs["u3"], np.float32))
    for bn in ("b1", "b2", "b3"):
        assert not np.any(np.asarray(inputs[bn])), "nonzero conv bias unsupported"
    emb = np.asarray(inputs["emb"], np.float32)
    cond_w = np.asarray(inputs["cond_w"], np.float32)
    cond_b = np.asarray(inputs["cond_b"], np.float32)
    mb_w = np.ascontiguousarray(np.asarray(inputs["mb_w"], np.float32))
    fc_w = np.asarray(inputs["fc_w"], np.float32)
    fc_b = np.asarray(inputs["fc_b"], np.float32)

    # conv1 im2col: rows (ci*5 + t), cols (n*322 + l): x[n, 2l+t-4, ci]
    xpad = np.zeros((N, L + 8, LEADS), np.float32)
    xpad[:, 4 : 4 + L, :] = ecg
    im2 = np.empty((15, N, L1), np.float32)
    for ci in range(LEADS):
        for t in range(5):
            im2[ci * 5 + t] = xpad[:, t : t + 2 * L1 : 2, ci]
    w1i = np.ascontiguousarray(w1.transpose(1, 2, 0).reshape(15, 64))

    w2p = np.zeros((128, 4, 128), np.float32)
    for p in range(4):
        w2p[0:64, p, :] = w2[:, :, 2 * p].T
        if 2 * p + 1 < 7:
            w2p[64:128, p, :] = w2[:, :, 2 * p + 1].T
    w2p = w2p.reshape(128, 512).astype(ml_dtypes.bfloat16)

    w3t = np.ascontiguousarray(w3.transpose(1, 2, 0).reshape(128, 2304))

    sel = np.zeros((125, 32), np.float32)
    for kd in range(125):
        sel[kd, kd // 5] = 1.0
    sel = sel.astype(ml_dtypes.bfloat16)

    fcwf = np.zeros((128, 2), np.float32)
    for c in range(4):
        fcwf[32 * c : 32 * c + 25, 0] = fc_w[0, FLAT + 25 * c : FLAT + 25 * c + 25]

    mbw_aug = np.concatenate(
        [mb_w, np.stack([fc_w[0, :FLAT], fc_w[0, :FLAT]], axis=1)], axis=1
    ).astype(np.float32)
    mbw_aug = np.ascontiguousarray(mbw_aug)

    cemb = emb[condition[:, 0]]
    cond = cemb @ cond_w.T + cond_b
    condc = (cond @ fc_w[0, FLAT + 100 :] + fc_b[0]).astype(np.float32)  # [512]

    in_maps = []
    for c in range(NC):
        rows = slice(c * NB, (c + 1) * NB)
        in_maps.append({
            "im2": np.ascontiguousarray(im2[:, rows, :].reshape(15, NB * L1)),
            "w1i": w1i,
            "w2p": w2p,
            "w3t": w3t,
            "sel": sel,
            "fcwf": fcwf,
            "mbw": mbw_aug,
            "condc": np.ascontiguousarray(condc[rows].reshape(NB, 1)),
            "zeros": np.zeros((128, 256), np.float32),
        })
    return in_maps


def _run(inputs, trace=False):
    if "nc" not in _STATE:
        _STATE["nc"] = _build_program()
    in_maps = _host_prep(inputs)
    res = run_bass_kernel_spmd(_STATE["nc"], in_maps, list(range(NC)), trace=trace)
    out = np.concatenate([res.results[c]["y"] for c in range(NC)], axis=0)
    return out.astype(np.float32), res


def kernel(**inputs) -> np.ndarray:
    out, _ = _run(inputs, trace=False)
    return out
